# revision 20
# baseline (speedup 1.0000x reference)
"""Causal GQA self-attention (B=4, T=2048, C=2048, 16 Q heads / 8 KV heads,
hd=128) as a Bass/Tile SPMD kernel on 8 Trainium2 NeuronCores.

Sharding: core c = (batch b = c//2, head-group g = c%2). Each core handles one
batch and 8 Q heads / 4 KV heads. Wq/Wk/Wv column-sharded on the head dim, Wo
row-sharded; the host sums the two partial Wo products per batch.

All on-device tensors live in a transposed [feature, token] layout so every
matmul contraction sits on the partition dim with no on-device transposes.
Everything runs bf16 with fp32 PSUM accumulation (~4.4e-3 end-to-end rel err).

Performance design (1163us -> ~514us, PE ~95% busy at the 2.4GHz warm clock):
- All weights are SBUF-resident (96KB/partition), loaded once up front via 16
  wide DMAs; streaming them per block saturated the Sync engine's ~600ns
  DIRECT2D dispatch path and left the PE clock-throttled waiting on weights.
- Exact causal masking: diagonal-block score/exp/attnV work only touches the
  valid [lo:TB] query-column range; one 128x128 triangular multiply per
  diagonal sub-block.
- Softmax denominator: groups of 4 exp tiles are folded on the DVE, then one
  accumulating ones-matmul per group with a [128,128] all-ones stationary
  produces the denominator already broadcast across partitions, so the
  normalization tail is just DVE reciprocal + one multiply (no GpSimd hop,
  no [1,512] matmuls burning PE streams).
- Software pipeline: attention(tb-1) is emitted before proj(tb), and Wo is
  delayed a further block (wo(tb-2)), so the readiness-driven Tile scheduler
  always has dense projection/Wo matmuls to fill the ACT-paced gaps of the
  attention phase.
- PSUM: proj/Wo chains are single-bank (2-bank ping-pong), scores 3 banks,
  out-accum 2, denominator 1 = exactly 8 banks.
- PSUM->SBUF casts (V tiles, y tiles, rope staging) run on the ScalarE, which
  is otherwise idle between exp bursts; keeping them off the DVE removes
  head-of-line blocking of the norm multiplies that gate Wo.
- 64 dummy matmuls on const tiles during the startup DMA window trip the
  PE_HAM activity monitor to the full 2.4GHz clock before real work arrives.
- y is written bf16; the host sums the two partial Wo products per batch.
"""

import sys

import ml_dtypes
import numpy as np

sys.path.insert(0, "/opt/trn_rl_repo")

import concourse.bass as bass  # noqa: E402
import concourse.mybir as mybir  # noqa: E402
import concourse.tile as tile  # noqa: E402
from concourse import bacc  # noqa: E402
from concourse.bass_utils import run_bass_kernel_spmd  # noqa: E402

# Problem shape (hardcoded per contest contract).
B = 4
T = 2048
C = 2048
HD = 128
N_HEAD = 16
N_KV_HEAD = 8
NQH = N_HEAD // 2  # q heads per core (group)
NKV = N_KV_HEAD // 2  # kv heads per core
TB = 512  # token block
NTB = T // TB
NCT = C // 128  # contraction tiles for the projections
SCALE = 1.0 / float(np.sqrt(HD))

F32 = mybir.dt.float32
BF16 = mybir.dt.bfloat16
MULT = mybir.AluOpType.mult
ADD = mybir.AluOpType.add
EXP = mybir.ActivationFunctionType.Exp


def _rope(nc, tmpp, dst, src_psum, cosb, nsinb):
    """dst = src*cos + rot_half(src)*sin, src in [d, t] layout (d partitions).

    rot_half(x)[d] = -x[d+64] for d<64, +x[d-64] for d>=64; the sign lives in
    nsinb so both halves are plain multiplies. nsinb is the sin table rotated
    by 64 partitions (nsinb[64+i] = -sin[i], nsinb[i] = sin[64+i]) so each
    tensor_tensor has equal base partitions on its two SBUF inputs (HW rule).
    All SBUF-side math is bf16 so the DVE runs in its 2x packed mode.
    """
    t0 = tmpp.tile([HD, TB], BF16, tag="t0")
    nc.scalar.copy(t0[:], src_psum[:])
    nc.vector.tensor_mul(dst, t0[:], cosb[:])
    t2 = tmpp.tile([HD, TB], BF16, tag="t2")
    nc.vector.tensor_mul(t2[0:64, :], t0[64:128, :], nsinb[64:128, :])
    nc.vector.tensor_mul(t2[64:128, :], t0[0:64, :], nsinb[0:64, :])
    nc.vector.scalar_tensor_tensor(dst, t2[:], 1.0, dst, op0=MULT, op1=ADD)


def build_nc():
    nc = bacc.Bacc("TRN2", target_bir_lowering=False, debug=False, num_devices=8)

    xT = nc.dram_tensor("xT", [C, T], BF16, kind="ExternalInput")
    wqT = nc.dram_tensor("wqT", [C, NQH * HD], BF16, kind="ExternalInput")
    wkT = nc.dram_tensor("wkT", [C, NKV * HD], BF16, kind="ExternalInput")
    wvT = nc.dram_tensor("wvT", [C, NKV * HD], BF16, kind="ExternalInput")
    woT = nc.dram_tensor("woT", [NQH * HD, C], BF16, kind="ExternalInput")
    cosdt = nc.dram_tensor("cosdt", [HD, T], BF16, kind="ExternalInput")
    nsindt = nc.dram_tensor("nsindt", [HD, T], BF16, kind="ExternalInput")
    tridt = nc.dram_tensor("tridt", [128, 128], BF16, kind="ExternalInput")
    onescol = nc.dram_tensor("onescol", [128, 128], BF16, kind="ExternalInput")
    yT = nc.dram_tensor("yT", [C, T], BF16, kind="ExternalOutput")

    from contextlib import ExitStack

    with ExitStack() as es:
        tc = es.enter_context(tile.TileContext(nc))
        es.enter_context(nc.allow_low_precision("bf16 attention"))
        constp = es.enter_context(tc.tile_pool(name="const", bufs=1))
        strp = es.enter_context(tc.tile_pool(name="stream", bufs=2))
        perp = es.enter_context(tc.tile_pool(name="persist", bufs=1))
        xp = es.enter_context(tc.tile_pool(name="xp", bufs=6))
        qp = es.enter_context(tc.tile_pool(name="qt", bufs=16))
        outp = es.enter_context(tc.tile_pool(name="ot", bufs=16))
        tmpp = es.enter_context(tc.tile_pool(name="tmp", bufs=2))
        expp = es.enter_context(tc.tile_pool(name="exps", bufs=6))
        foldp = es.enter_context(tc.tile_pool(name="fold", bufs=2))
        smallp = es.enter_context(tc.tile_pool(name="small", bufs=2))
        yp = es.enter_context(tc.tile_pool(name="ysb", bufs=2))
        projp = es.enter_context(tc.tile_pool(name="pp", bufs=2, space="PSUM"))
        spsum = es.enter_context(tc.tile_pool(name="sp", bufs=3, space="PSUM"))
        opsum = es.enter_context(tc.tile_pool(name="op", bufs=2, space="PSUM"))
        denp = es.enter_context(tc.tile_pool(name="dn", bufs=1, space="PSUM"))
        if True:
            tri = constp.tile([128, 128], BF16, tag="tri")
            nc.sync.dma_start(tri[:], tridt[:])
            ones_c = constp.tile([128, 128], BF16, tag="onesc")
            nc.sync.dma_start(ones_c[:], onescol[:])

            # ~7us of dummy matmuls during the startup DMA window: trips the
            # PE_HAM activity monitor to full clock (4096-cycle window) so the
            # first real projection chains run at 2.4GHz instead of 1.2GHz
            warm = spsum.tile([128, 128], F32, tag="sp", name="warmup")
            for _ in range(64):
                nc.tensor.matmul(warm[:], tri[:], ones_c[:], start=True, stop=True)

            # Resident weights, 4 wide DMAs per matrix (Sync DIRECT2D dispatch
            # costs ~600ns per dma_start; 16 dispatches instead of 56). Each
            # wide tile packs 4 contraction groups side by side.
            def _load_wide(dram, rows, width, tag):
                # dram is [4*rows_grp..], load [rows, width] per group, 4
                # groups per wide tile -> 4 tiles covering 16 row-groups
                tiles = []
                for j in range(4):
                    t_ = constp.tile([128, 4 * width], BF16, tag=f"{tag}{j}")
                    src = dram[j * 512 : (j + 1) * 512, :].rearrange(
                        "(c p) t -> p c t", c=4, p=128
                    )
                    nc.sync.dma_start(
                        t_[:].rearrange("p (c t) -> p c t", c=4, t=width), src
                    )
                    tiles.append(t_)
                return tiles

            wk_sb, wv_sb, wq_sb, wo_sb = [], [], [], []

            def emit_wk():
                wk_w = _load_wide(wkT, 128, NKV * HD, "wk")
                wk_sb.extend(wk_w[ct // 4][:, (ct % 4) * 512 : (ct % 4 + 1) * 512] for ct in range(NCT))

            def emit_weight_loads():
                # Emitted AFTER block 0's x/cos DMAs so block-0 data is at the
                # front of the Sync dispatch queue and the first K-proj chain
                # starts as early as possible.
                wv_w = _load_wide(wvT, 128, NKV * HD, "wv")
                wq_w = _load_wide(wqT, 128, NQH * HD, "wq")
                # wo: 8 row-groups of [128, 2048] -> 4 tiles of 2 groups each
                wo_w = []
                for j in range(4):
                    t_ = constp.tile([128, 2 * C], BF16, tag=f"wo{j}")
                    src = woT[j * 256 : (j + 1) * 256, :].rearrange(
                        "(c p) t -> p c t", c=2, p=128
                    )
                    nc.sync.dma_start(
                        t_[:].rearrange("p (c t) -> p c t", c=2, t=C), src
                    )
                    wo_w.append(t_)
                wv_sb.extend(wv_w[ct // 4][:, (ct % 4) * 512 : (ct % 4 + 1) * 512] for ct in range(NCT))
                wq_sb.extend(wq_w[ct // 4][:, (ct % 4) * 1024 : (ct % 4 + 1) * 1024] for ct in range(NCT))
                wo_sb.extend(wo_w[jh // 2][:, (jh % 2) * 2048 : (jh % 2 + 1) * 2048] for jh in range(NQH))

            kT = [perp.tile([HD, T], BF16, tag=f"kT{h}", name=f"kT{h}") for h in range(NKV)]
            vT = [perp.tile([128, NKV * HD], BF16, tag=f"v{i}", name=f"v{i}") for i in range(T // 128)]

            def load_block(tb, weight_hook=None):
                tsl = slice(tb * TB, (tb + 1) * TB)
                # x lands in 4 wide tiles (4 contraction groups each) so the
                # Sync engine dispatches 4 DMAs per block instead of 16
                xw = []
                for j in range(4):
                    t_ = xp.tile([128, 4 * TB], BF16, tag="xb", name=f"xb{tb}_{j}")
                    src = xT[j * 512 : (j + 1) * 512, tsl].rearrange(
                        "(c p) t -> p c t", c=4, p=128
                    )
                    dst = t_[:].rearrange("p (c t) -> p c t", c=4, t=TB)
                    nc.sync.dma_start(dst, src)
                    xw.append(t_)
                    if j == 0 and weight_hook is not None:
                        # block 0: dispatch wk right after x[j0] so the first
                        # K-proj chain's inputs are at the queue front
                        weight_hook()
                xb = [
                    xw[ct // 4][:, (ct % 4) * TB : (ct % 4 + 1) * TB]
                    for ct in range(NCT)
                ]
                cosb = strp.tile([HD, TB], BF16, tag="cosb", name=f"cosb{tb}")
                nc.sync.dma_start(cosb[:], cosdt[:, tsl])
                nsinb = strp.tile([HD, TB], BF16, tag="nsinb", name=f"nsinb{tb}")
                nc.sync.dma_start(nsinb[:], nsindt[:, tsl])
                return xb, cosb, nsinb

            def proj_block(tb, xb, cosb, nsinb):
                tsl = slice(tb * TB, (tb + 1) * TB)
                # K projection (k^T layout [d, t]) + RoPE; one PSUM bank/chain
                for h in range(NKV):
                    kps = projp.tile([128, TB], F32, tag="pp", name=f"kps{tb}_{h}")
                    for ct in range(NCT):
                        nc.tensor.matmul(
                            kps[:],
                            wk_sb[ct][:, h * 128 : (h + 1) * 128],
                            xb[ct][:],
                            start=(ct == 0),
                            stop=(ct == NCT - 1),
                        )
                    _rope(nc, tmpp, kT[h][:, tsl], kps, cosb, nsinb)

                # V projection in [t, d] layout; one chain per 128-token slab
                for i in range(4):
                    vps = projp.tile([128, NKV * HD], F32, tag="pp", name=f"vps{tb}_{i}")
                    for ct in range(NCT):
                        nc.tensor.matmul(
                            vps[:],
                            xb[ct][:, i * 128 : (i + 1) * 128],
                            wv_sb[ct][:],
                            start=(ct == 0),
                            stop=(ct == NCT - 1),
                        )
                    nc.scalar.copy(vT[4 * tb + i][:], vps[:])

                # Q projection (q^T layout) + RoPE
                qts = []
                for h in range(NQH):
                    qps = projp.tile([128, TB], F32, tag="pp", name=f"qps{tb}_{h}")
                    for ct in range(NCT):
                        nc.tensor.matmul(
                            qps[:],
                            wq_sb[ct][:, h * 128 : (h + 1) * 128],
                            xb[ct][:],
                            start=(ct == 0),
                            stop=(ct == NCT - 1),
                        )
                    qt = qp.tile([HD, TB], BF16, tag="qt", name=f"qt{tb}_{h}")
                    _rope(nc, tmpp, qt[:], qps, cosb, nsinb)
                    qts.append(qt)
                return qts

            def attention_block(tb, qts):
                ktmax = 4 * tb + 4
                ngrp = tb + 1  # den fold groups of 4 kt tiles
                outs = []
                for h in range(NQH):
                    hv = h // 2
                    ops_ = opsum.tile([HD, TB], F32, tag="op", name=f"aop{tb}_{h}")
                    den = denp.tile([128, TB], F32, tag="dn", name=f"den{tb}_{h}")
                    fold = None
                    ex0 = None
                    for kt in range(ktmax):
                        # exact causal: the diagonal-block kt tiles only touch
                        # query columns >= the key tile start, so scores / exp /
                        # attnV / den all run on the [lo:TB] column range
                        m = kt - 4 * tb
                        lo = 128 * m if m > 0 else 0
                        sps = spsum.tile([128, TB], F32, tag="sp")
                        nc.tensor.matmul(
                            sps[:, lo:TB],
                            kT[hv][:, kt * 128 : (kt + 1) * 128],
                            qts[h][:, lo:TB],
                            start=True,
                            stop=True,
                        )
                        ex = expp.tile([128, TB], BF16, tag="exps")
                        nc.scalar.activation(ex[:, lo:TB], sps[:, lo:TB], EXP, scale=SCALE)
                        if m >= 0:
                            # triangular mask on the diagonal 128x128 sub-block
                            nc.vector.tensor_mul(
                                ex[:, 128 * m : 128 * (m + 1)],
                                ex[:, 128 * m : 128 * (m + 1)],
                                tri[:],
                            )
                        nc.tensor.matmul(
                            ops_[:, lo:TB],
                            vT[kt][:, hv * 128 : (hv + 1) * 128],
                            ex[:, lo:TB],
                            start=(kt == 0),
                            stop=(kt == ktmax - 1),
                        )
                        # denominator: fold 4 exp tiles on the DVE, then one
                        # accumulating ones-matmul per group on the PE
                        r = kt % 4
                        if r == 0:
                            ex0 = ex
                        elif r == 1:
                            fold = foldp.tile([128, TB], BF16, tag="fold")
                            if m == 1:
                                # diagonal group: seed with a fast 4x copy,
                                # then range-limited adds
                                nc.vector.tensor_copy(fold[:], ex0[:])
                                nc.vector.tensor_add(
                                    fold[:, lo:TB], fold[:, lo:TB], ex[:, lo:TB]
                                )
                            else:
                                nc.vector.tensor_add(fold[:], ex0[:], ex[:])
                        else:
                            nc.vector.tensor_add(
                                fold[:, lo:TB], fold[:, lo:TB], ex[:, lo:TB]
                            )
                            if r == 3:
                                g = kt // 4
                                nc.tensor.matmul(
                                    den[:],
                                    ones_c[:],
                                    fold[:],
                                    start=(g == 0),
                                    stop=(g == ngrp - 1),
                                )
                    # den was produced by a [128,128] all-ones stationary so
                    # it is already partition-broadcast; single-op approx
                    # reciprocal (~18 bits, plenty) then one DVE multiply.
                    # (v2 bounced through a GpSimd PartitionBroadcast that
                    # stalled block boundaries by multiple us)
                    rec = smallp.tile([128, TB], F32, tag="rec")
                    nc.vector.reciprocal_approx_fast(rec[:], den[:])
                    ot = outp.tile([HD, TB], BF16, tag="ot")
                    nc.vector.tensor_mul(ot[:], ops_[:], rec[:])
                    outs.append(ot)
                return outs

            def wo_block(tb, outs):
                tsl = slice(tb * TB, (tb + 1) * TB)
                for og in range(16):
                    yps = projp.tile([128, TB], F32, tag="pp", name=f"yps{tb}_{og}")
                    for jh in range(NQH):
                        nc.tensor.matmul(
                            yps[:],
                            wo_sb[jh][:, og * 128 : (og + 1) * 128],
                            outs[jh][:],
                            start=(jh == 0),
                            stop=(jh == NQH - 1),
                        )
                    ysb = yp.tile([128, TB], BF16, tag="ysb")
                    nc.scalar.copy(ysb[:], yps[:])
                    nc.sync.dma_start(yT[og * 128 : (og + 1) * 128, tsl], ysb[:])

            # Software pipeline: attention of block tb-1 is emitted BEFORE the
            # projections of block tb so the ACT-gated attention phase always
            # has dense projection matmuls to fill PE gaps; Wo of tb-1 goes
            # AFTER proj(tb) so its long outs[7] dependency chain (den->recip->
            # broadcast->norm) resolves while proj matmuls keep the PE warm.
            # Wo is additionally delayed one block (wo(tb-2) emitted at
            # iteration tb, wo(2) after the final attention) so every
            # ACT-paced attention phase has a ready pool of dense Wo matmuls
            # for the scheduler to fill PE stalls with.
            prev_qts = None
            prev_outs = None
            for tb in range(NTB):
                xb, cosb, nsinb = load_block(tb, weight_hook=emit_wk if tb == 0 else None)
                if tb == 0:
                    emit_weight_loads()
                outs = None
                if prev_qts is not None:
                    outs = attention_block(tb - 1, prev_qts)
                prev_qts = proj_block(tb, xb, cosb, nsinb)
                if prev_outs is not None:
                    wo_block(tb - 2, prev_outs)
                prev_outs = outs
            outs = attention_block(NTB - 1, prev_qts)
            wo_block(NTB - 2, prev_outs)
            wo_block(NTB - 1, outs)

    nc.compile()
    return nc


def _host_consts():
    inv_freq = 1.0 / (10000.0 ** (np.arange(0, HD, 2, dtype=np.float32) / HD))
    t = np.arange(T, dtype=np.float32)
    freqs = np.outer(t, inv_freq)  # [T, HD/2]
    freqs = np.repeat(freqs, 2, axis=-1)  # [T, HD]
    bf = ml_dtypes.bfloat16
    cos = np.cos(freqs).astype(np.float32).T.copy()  # [HD, T]
    sin = np.sin(freqs).astype(np.float32).T.copy()
    # rotated-by-64 signed sin table: row d holds the multiplier that pairs
    # with x[(d+64)%128]; rows 64..127 carry -sin[0:64], rows 0..63 +sin[64:128]
    nsin = np.empty_like(sin)
    nsin[0:64, :] = sin[64:128, :]
    nsin[64:128, :] = -sin[0:64, :]

    kp = np.arange(128)[:, None]
    qf = np.arange(128)[None, :]
    tri = (kp <= qf).astype(bf)

    return {
        "cosdt": np.ascontiguousarray(cos.astype(bf)),
        "nsindt": np.ascontiguousarray(nsin.astype(bf)),
        "tridt": tri,
        "onescol": np.ones((128, 128), dtype=bf),
    }


_NC_CACHE = None


def _get_nc():
    global _NC_CACHE
    if _NC_CACHE is None:
        _NC_CACHE = build_nc()
    return _NC_CACHE


def kernel(x, Wq, Wk, Wv, Wo, _trace=False):
    x = np.asarray(x, dtype=np.float32)
    Wq = np.asarray(Wq, dtype=np.float32)
    Wk = np.asarray(Wk, dtype=np.float32)
    Wv = np.asarray(Wv, dtype=np.float32)
    Wo = np.asarray(Wo, dtype=np.float32)

    nc = _get_nc()
    consts = _host_consts()

    bf = ml_dtypes.bfloat16
    xTs = [np.ascontiguousarray(x[b].T.astype(bf)) for b in range(B)]
    wqTs = [np.ascontiguousarray(Wq[1024 * g : 1024 * (g + 1), :].T.astype(bf)) for g in range(2)]
    wkTs = [np.ascontiguousarray(Wk[512 * g : 512 * (g + 1), :].T.astype(bf)) for g in range(2)]
    wvTs = [np.ascontiguousarray(Wv[512 * g : 512 * (g + 1), :].T.astype(bf)) for g in range(2)]
    woTs = [np.ascontiguousarray(Wo[:, 1024 * g : 1024 * (g + 1)].T.astype(bf)) for g in range(2)]

    in_maps = []
    for c in range(8):
        b, g = c // 2, c % 2
        im = {
            "xT": xTs[b],
            "wqT": wqTs[g],
            "wkT": wkTs[g],
            "wvT": wvTs[g],
            "woT": woTs[g],
        }
        im.update(consts)
        in_maps.append(im)

    res = run_bass_kernel_spmd(nc, in_maps, core_ids=list(range(8)), trace=_trace)

    y = np.empty((B, T, C), dtype=np.float32)
    for b in range(B):
        y[b] = (
            res.results[2 * b]["yT"].astype(np.float32)
            + res.results[2 * b + 1]["yT"].astype(np.float32)
        ).T
    if _trace:
        return y, res
    return y


# revision 21
# speedup vs baseline: 1.0029x; 1.0029x over previous
"""Causal GQA self-attention (B=4, T=2048, C=2048, 16 Q heads / 8 KV heads,
hd=128) as a Bass/Tile SPMD kernel on 8 Trainium2 NeuronCores.

Sharding: core c = (batch b = c//2, head-group g = c%2). Each core handles one
batch and 8 Q heads / 4 KV heads. Wq/Wk/Wv column-sharded on the head dim, Wo
row-sharded; the host sums the two partial Wo products per batch.

All on-device tensors live in a transposed [feature, token] layout so every
matmul contraction sits on the partition dim with no on-device transposes.
Everything runs bf16 with fp32 PSUM accumulation (~4.4e-3 end-to-end rel err).

Performance design (1163us -> ~514us, PE ~95% busy at the 2.4GHz warm clock):
- All weights are SBUF-resident (96KB/partition), loaded once up front via 16
  wide DMAs; streaming them per block saturated the Sync engine's ~600ns
  DIRECT2D dispatch path and left the PE clock-throttled waiting on weights.
- Exact causal masking: diagonal-block score/exp/attnV work only touches the
  valid [lo:TB] query-column range; one 128x128 triangular multiply per
  diagonal sub-block.
- Softmax denominator: groups of 4 exp tiles are folded on the DVE, then one
  accumulating ones-matmul per group with a [128,128] all-ones stationary
  produces the denominator already broadcast across partitions, so the
  normalization tail is just DVE reciprocal + one multiply (no GpSimd hop,
  no [1,512] matmuls burning PE streams).
- Software pipeline: attention(tb-1) is emitted before proj(tb), and Wo is
  delayed a further block (wo(tb-2)), so the readiness-driven Tile scheduler
  always has dense projection/Wo matmuls to fill the ACT-paced gaps of the
  attention phase.
- PSUM: proj/Wo chains are single-bank (2-bank ping-pong), scores 3 banks,
  out-accum 2, denominator 1 = exactly 8 banks.
- PSUM->SBUF casts (V tiles, y tiles, rope staging) run on the ScalarE, which
  is otherwise idle between exp bursts; keeping them off the DVE removes
  head-of-line blocking of the norm multiplies that gate Wo.
- 64 dummy matmuls on const tiles during the startup DMA window trip the
  PE_HAM activity monitor to the full 2.4GHz clock before real work arrives.
- y is written bf16; the host sums the two partial Wo products per batch.
"""

import sys

import ml_dtypes
import numpy as np

sys.path.insert(0, "/opt/trn_rl_repo")

import concourse.bass as bass  # noqa: E402
import concourse.mybir as mybir  # noqa: E402
import concourse.tile as tile  # noqa: E402
from concourse import bacc  # noqa: E402
from concourse.bass_utils import run_bass_kernel_spmd  # noqa: E402

# Problem shape (hardcoded per contest contract).
B = 4
T = 2048
C = 2048
HD = 128
N_HEAD = 16
N_KV_HEAD = 8
NQH = N_HEAD // 2  # q heads per core (group)
NKV = N_KV_HEAD // 2  # kv heads per core
TB = 512  # token block
NTB = T // TB
NCT = C // 128  # contraction tiles for the projections
SCALE = 1.0 / float(np.sqrt(HD))

F32 = mybir.dt.float32
BF16 = mybir.dt.bfloat16
MULT = mybir.AluOpType.mult
ADD = mybir.AluOpType.add
EXP = mybir.ActivationFunctionType.Exp


def _rope(nc, tmpp, dst, src_psum, cosb, nsinb):
    """dst = src*cos + rot_half(src)*sin, src in [d, t] layout (d partitions).

    rot_half(x)[d] = -x[d+64] for d<64, +x[d-64] for d>=64; the sign lives in
    nsinb so both halves are plain multiplies. nsinb is the sin table rotated
    by 64 partitions (nsinb[64+i] = -sin[i], nsinb[i] = sin[64+i]) so each
    tensor_tensor has equal base partitions on its two SBUF inputs (HW rule).
    All SBUF-side math is bf16 so the DVE runs in its 2x packed mode.
    """
    t0 = tmpp.tile([HD, TB], BF16, tag="t0")
    nc.scalar.copy(t0[:], src_psum[:])
    nc.vector.tensor_mul(dst, t0[:], cosb[:])
    t2 = tmpp.tile([HD, TB], BF16, tag="t2")
    nc.vector.tensor_mul(t2[0:64, :], t0[64:128, :], nsinb[64:128, :])
    nc.vector.tensor_mul(t2[64:128, :], t0[0:64, :], nsinb[0:64, :])
    nc.vector.scalar_tensor_tensor(dst, t2[:], 1.0, dst, op0=MULT, op1=ADD)


def build_nc():
    nc = bacc.Bacc("TRN2", target_bir_lowering=False, debug=False, num_devices=8)

    xT = nc.dram_tensor("xT", [C, T], BF16, kind="ExternalInput")
    wqT = nc.dram_tensor("wqT", [C, NQH * HD], BF16, kind="ExternalInput")
    wkT = nc.dram_tensor("wkT", [C, NKV * HD], BF16, kind="ExternalInput")
    wvT = nc.dram_tensor("wvT", [C, NKV * HD], BF16, kind="ExternalInput")
    woT = nc.dram_tensor("woT", [NQH * HD, C], BF16, kind="ExternalInput")
    cosdt = nc.dram_tensor("cosdt", [HD, T], BF16, kind="ExternalInput")
    nsindt = nc.dram_tensor("nsindt", [HD, T], BF16, kind="ExternalInput")
    tridt = nc.dram_tensor("tridt", [128, 128], BF16, kind="ExternalInput")
    onescol = nc.dram_tensor("onescol", [128, 128], BF16, kind="ExternalInput")
    yT = nc.dram_tensor("yT", [C, T], BF16, kind="ExternalOutput")

    from contextlib import ExitStack

    with ExitStack() as es:
        tc = es.enter_context(tile.TileContext(nc))
        es.enter_context(nc.allow_low_precision("bf16 attention"))
        constp = es.enter_context(tc.tile_pool(name="const", bufs=1))
        strp = es.enter_context(tc.tile_pool(name="stream", bufs=2))
        perp = es.enter_context(tc.tile_pool(name="persist", bufs=1))
        xp = es.enter_context(tc.tile_pool(name="xp", bufs=6))
        qp = es.enter_context(tc.tile_pool(name="qt", bufs=16))
        outp = es.enter_context(tc.tile_pool(name="ot", bufs=16))
        tmpp = es.enter_context(tc.tile_pool(name="tmp", bufs=2))
        expp = es.enter_context(tc.tile_pool(name="exps", bufs=6))
        foldp = es.enter_context(tc.tile_pool(name="fold", bufs=2))
        smallp = es.enter_context(tc.tile_pool(name="small", bufs=2))
        yp = es.enter_context(tc.tile_pool(name="ysb", bufs=2))
        projp = es.enter_context(tc.tile_pool(name="pp", bufs=2, space="PSUM"))
        spsum = es.enter_context(tc.tile_pool(name="sp", bufs=3, space="PSUM"))
        opsum = es.enter_context(tc.tile_pool(name="op", bufs=2, space="PSUM"))
        denp = es.enter_context(tc.tile_pool(name="dn", bufs=1, space="PSUM"))
        if True:
            tri = constp.tile([128, 128], BF16, tag="tri")
            nc.sync.dma_start(tri[:], tridt[:])
            ones_c = constp.tile([128, 128], BF16, tag="onesc")
            nc.sync.dma_start(ones_c[:], onescol[:])

            # ~5us of dummy matmuls during the startup DMA window: trips the
            # PE_HAM activity monitor to full clock (4096-cycle window) so the
            # first real projection chains run at 2.4GHz instead of 1.2GHz
            warm = spsum.tile([128, 128], F32, tag="sp", name="warmup")
            for _ in range(44):
                nc.tensor.matmul(warm[:], tri[:], ones_c[:], start=True, stop=True)

            # Resident weights, 4 wide DMAs per matrix (Sync DIRECT2D dispatch
            # costs ~600ns per dma_start; 16 dispatches instead of 56). Each
            # wide tile packs 4 contraction groups side by side.
            def _load_wide(dram, rows, width, tag):
                # dram is [4*rows_grp..], load [rows, width] per group, 4
                # groups per wide tile -> 4 tiles covering 16 row-groups
                tiles = []
                for j in range(4):
                    t_ = constp.tile([128, 4 * width], BF16, tag=f"{tag}{j}")
                    src = dram[j * 512 : (j + 1) * 512, :].rearrange(
                        "(c p) t -> p c t", c=4, p=128
                    )
                    nc.sync.dma_start(
                        t_[:].rearrange("p (c t) -> p c t", c=4, t=width), src
                    )
                    tiles.append(t_)
                return tiles

            wk_sb, wv_sb, wq_sb, wo_sb = [], [], [], []

            def emit_wk():
                wk_w = _load_wide(wkT, 128, NKV * HD, "wk")
                wk_sb.extend(wk_w[ct // 4][:, (ct % 4) * 512 : (ct % 4 + 1) * 512] for ct in range(NCT))

            def emit_weight_loads():
                # Emitted AFTER block 0's x/cos DMAs so block-0 data is at the
                # front of the Sync dispatch queue and the first K-proj chain
                # starts as early as possible.
                wv_w = _load_wide(wvT, 128, NKV * HD, "wv")
                wq_w = _load_wide(wqT, 128, NQH * HD, "wq")
                # wo: 8 row-groups of [128, 2048] -> 4 tiles of 2 groups each
                wo_w = []
                for j in range(4):
                    t_ = constp.tile([128, 2 * C], BF16, tag=f"wo{j}")
                    src = woT[j * 256 : (j + 1) * 256, :].rearrange(
                        "(c p) t -> p c t", c=2, p=128
                    )
                    nc.sync.dma_start(
                        t_[:].rearrange("p (c t) -> p c t", c=2, t=C), src
                    )
                    wo_w.append(t_)
                wv_sb.extend(wv_w[ct // 4][:, (ct % 4) * 512 : (ct % 4 + 1) * 512] for ct in range(NCT))
                wq_sb.extend(wq_w[ct // 4][:, (ct % 4) * 1024 : (ct % 4 + 1) * 1024] for ct in range(NCT))
                wo_sb.extend(wo_w[jh // 2][:, (jh % 2) * 2048 : (jh % 2 + 1) * 2048] for jh in range(NQH))

            kT = [perp.tile([HD, T], BF16, tag=f"kT{h}", name=f"kT{h}") for h in range(NKV)]
            vT = [perp.tile([128, NKV * HD], BF16, tag=f"v{i}", name=f"v{i}") for i in range(T // 128)]

            def load_block(tb, weight_hook=None):
                tsl = slice(tb * TB, (tb + 1) * TB)
                # x lands in 4 wide tiles (4 contraction groups each) so the
                # Sync engine dispatches 4 DMAs per block instead of 16
                xw = []
                for j in range(4):
                    t_ = xp.tile([128, 4 * TB], BF16, tag="xb", name=f"xb{tb}_{j}")
                    src = xT[j * 512 : (j + 1) * 512, tsl].rearrange(
                        "(c p) t -> p c t", c=4, p=128
                    )
                    dst = t_[:].rearrange("p (c t) -> p c t", c=4, t=TB)
                    nc.sync.dma_start(dst, src)
                    xw.append(t_)
                    if j == 0 and weight_hook is not None:
                        # block 0: dispatch wk right after x[j0] so the first
                        # K-proj chain's inputs are at the queue front
                        weight_hook()
                xb = [
                    xw[ct // 4][:, (ct % 4) * TB : (ct % 4 + 1) * TB]
                    for ct in range(NCT)
                ]
                cosb = strp.tile([HD, TB], BF16, tag="cosb", name=f"cosb{tb}")
                nc.sync.dma_start(cosb[:], cosdt[:, tsl])
                nsinb = strp.tile([HD, TB], BF16, tag="nsinb", name=f"nsinb{tb}")
                nc.sync.dma_start(nsinb[:], nsindt[:, tsl])
                return xb, cosb, nsinb

            def proj_block(tb, xb, cosb, nsinb):
                tsl = slice(tb * TB, (tb + 1) * TB)
                # K projection (k^T layout [d, t]) + RoPE; one PSUM bank/chain
                for h in range(NKV):
                    kps = projp.tile([128, TB], F32, tag="pp", name=f"kps{tb}_{h}")
                    for ct in range(NCT):
                        nc.tensor.matmul(
                            kps[:],
                            wk_sb[ct][:, h * 128 : (h + 1) * 128],
                            xb[ct][:],
                            start=(ct == 0),
                            stop=(ct == NCT - 1),
                        )
                    _rope(nc, tmpp, kT[h][:, tsl], kps, cosb, nsinb)

                # V projection in [t, d] layout; one chain per 128-token slab
                for i in range(4):
                    vps = projp.tile([128, NKV * HD], F32, tag="pp", name=f"vps{tb}_{i}")
                    for ct in range(NCT):
                        nc.tensor.matmul(
                            vps[:],
                            xb[ct][:, i * 128 : (i + 1) * 128],
                            wv_sb[ct][:],
                            start=(ct == 0),
                            stop=(ct == NCT - 1),
                        )
                    nc.scalar.copy(vT[4 * tb + i][:], vps[:])

                # Q projection (q^T layout) + RoPE
                qts = []
                for h in range(NQH):
                    qps = projp.tile([128, TB], F32, tag="pp", name=f"qps{tb}_{h}")
                    for ct in range(NCT):
                        nc.tensor.matmul(
                            qps[:],
                            wq_sb[ct][:, h * 128 : (h + 1) * 128],
                            xb[ct][:],
                            start=(ct == 0),
                            stop=(ct == NCT - 1),
                        )
                    qt = qp.tile([HD, TB], BF16, tag="qt", name=f"qt{tb}_{h}")
                    _rope(nc, tmpp, qt[:], qps, cosb, nsinb)
                    qts.append(qt)
                return qts

            def attention_block(tb, qts):
                ktmax = 4 * tb + 4
                ngrp = tb + 1  # den fold groups of 4 kt tiles
                outs = []
                for h in range(NQH):
                    hv = h // 2
                    ops_ = opsum.tile([HD, TB], F32, tag="op", name=f"aop{tb}_{h}")
                    den = denp.tile([128, TB], F32, tag="dn", name=f"den{tb}_{h}")
                    fold = None
                    ex0 = None
                    for kt in range(ktmax):
                        # exact causal: the diagonal-block kt tiles only touch
                        # query columns >= the key tile start, so scores / exp /
                        # attnV / den all run on the [lo:TB] column range
                        m = kt - 4 * tb
                        lo = 128 * m if m > 0 else 0
                        sps = spsum.tile([128, TB], F32, tag="sp")
                        nc.tensor.matmul(
                            sps[:, lo:TB],
                            kT[hv][:, kt * 128 : (kt + 1) * 128],
                            qts[h][:, lo:TB],
                            start=True,
                            stop=True,
                        )
                        ex = expp.tile([128, TB], BF16, tag="exps")
                        nc.scalar.activation(ex[:, lo:TB], sps[:, lo:TB], EXP, scale=SCALE)
                        if m >= 0:
                            # triangular mask on the diagonal 128x128 sub-block
                            nc.vector.tensor_mul(
                                ex[:, 128 * m : 128 * (m + 1)],
                                ex[:, 128 * m : 128 * (m + 1)],
                                tri[:],
                            )
                        nc.tensor.matmul(
                            ops_[:, lo:TB],
                            vT[kt][:, hv * 128 : (hv + 1) * 128],
                            ex[:, lo:TB],
                            start=(kt == 0),
                            stop=(kt == ktmax - 1),
                        )
                        # denominator: fold 4 exp tiles on the DVE, then one
                        # accumulating ones-matmul per group on the PE
                        r = kt % 4
                        if r == 0:
                            ex0 = ex
                        elif r == 1:
                            fold = foldp.tile([128, TB], BF16, tag="fold")
                            if m == 1:
                                # diagonal group: seed with a fast 4x copy,
                                # then range-limited adds
                                nc.vector.tensor_copy(fold[:], ex0[:])
                                nc.vector.tensor_add(
                                    fold[:, lo:TB], fold[:, lo:TB], ex[:, lo:TB]
                                )
                            else:
                                nc.vector.tensor_add(fold[:], ex0[:], ex[:])
                        else:
                            nc.vector.tensor_add(
                                fold[:, lo:TB], fold[:, lo:TB], ex[:, lo:TB]
                            )
                            if r == 3:
                                g = kt // 4
                                nc.tensor.matmul(
                                    den[:],
                                    ones_c[:],
                                    fold[:],
                                    start=(g == 0),
                                    stop=(g == ngrp - 1),
                                )
                    # den was produced by a [128,128] all-ones stationary so
                    # it is already partition-broadcast; single-op approx
                    # reciprocal (~18 bits, plenty) then one DVE multiply.
                    # (v2 bounced through a GpSimd PartitionBroadcast that
                    # stalled block boundaries by multiple us)
                    rec = smallp.tile([128, TB], F32, tag="rec")
                    nc.vector.reciprocal_approx_fast(rec[:], den[:])
                    ot = outp.tile([HD, TB], BF16, tag="ot")
                    nc.vector.tensor_mul(ot[:], ops_[:], rec[:])
                    outs.append(ot)
                return outs

            def wo_block(tb, outs):
                tsl = slice(tb * TB, (tb + 1) * TB)
                for og in range(16):
                    yps = projp.tile([128, TB], F32, tag="pp", name=f"yps{tb}_{og}")
                    for jh in range(NQH):
                        nc.tensor.matmul(
                            yps[:],
                            wo_sb[jh][:, og * 128 : (og + 1) * 128],
                            outs[jh][:],
                            start=(jh == 0),
                            stop=(jh == NQH - 1),
                        )
                    ysb = yp.tile([128, TB], BF16, tag="ysb")
                    nc.scalar.copy(ysb[:], yps[:])
                    nc.sync.dma_start(yT[og * 128 : (og + 1) * 128, tsl], ysb[:])

            # Software pipeline: attention of block tb-1 is emitted BEFORE the
            # projections of block tb so the ACT-gated attention phase always
            # has dense projection matmuls to fill PE gaps; Wo of tb-1 goes
            # AFTER proj(tb) so its long outs[7] dependency chain (den->recip->
            # broadcast->norm) resolves while proj matmuls keep the PE warm.
            # Wo is additionally delayed one block (wo(tb-2) emitted at
            # iteration tb, wo(2) after the final attention) so every
            # ACT-paced attention phase has a ready pool of dense Wo matmuls
            # for the scheduler to fill PE stalls with.
            prev_qts = None
            prev_outs = None
            for tb in range(NTB):
                xb, cosb, nsinb = load_block(tb, weight_hook=emit_wk if tb == 0 else None)
                if tb == 0:
                    emit_weight_loads()
                outs = None
                if prev_qts is not None:
                    outs = attention_block(tb - 1, prev_qts)
                prev_qts = proj_block(tb, xb, cosb, nsinb)
                if prev_outs is not None:
                    wo_block(tb - 2, prev_outs)
                prev_outs = outs
            outs = attention_block(NTB - 1, prev_qts)
            wo_block(NTB - 2, prev_outs)
            wo_block(NTB - 1, outs)

    nc.compile()
    return nc


def _host_consts():
    inv_freq = 1.0 / (10000.0 ** (np.arange(0, HD, 2, dtype=np.float32) / HD))
    t = np.arange(T, dtype=np.float32)
    freqs = np.outer(t, inv_freq)  # [T, HD/2]
    freqs = np.repeat(freqs, 2, axis=-1)  # [T, HD]
    bf = ml_dtypes.bfloat16
    cos = np.cos(freqs).astype(np.float32).T.copy()  # [HD, T]
    sin = np.sin(freqs).astype(np.float32).T.copy()
    # rotated-by-64 signed sin table: row d holds the multiplier that pairs
    # with x[(d+64)%128]; rows 64..127 carry -sin[0:64], rows 0..63 +sin[64:128]
    nsin = np.empty_like(sin)
    nsin[0:64, :] = sin[64:128, :]
    nsin[64:128, :] = -sin[0:64, :]

    kp = np.arange(128)[:, None]
    qf = np.arange(128)[None, :]
    tri = (kp <= qf).astype(bf)

    return {
        "cosdt": np.ascontiguousarray(cos.astype(bf)),
        "nsindt": np.ascontiguousarray(nsin.astype(bf)),
        "tridt": tri,
        "onescol": np.ones((128, 128), dtype=bf),
    }


_NC_CACHE = None


def _get_nc():
    global _NC_CACHE
    if _NC_CACHE is None:
        _NC_CACHE = build_nc()
    return _NC_CACHE


def kernel(x, Wq, Wk, Wv, Wo, _trace=False):
    x = np.asarray(x, dtype=np.float32)
    Wq = np.asarray(Wq, dtype=np.float32)
    Wk = np.asarray(Wk, dtype=np.float32)
    Wv = np.asarray(Wv, dtype=np.float32)
    Wo = np.asarray(Wo, dtype=np.float32)

    nc = _get_nc()
    consts = _host_consts()

    bf = ml_dtypes.bfloat16
    xTs = [np.ascontiguousarray(x[b].T.astype(bf)) for b in range(B)]
    wqTs = [np.ascontiguousarray(Wq[1024 * g : 1024 * (g + 1), :].T.astype(bf)) for g in range(2)]
    wkTs = [np.ascontiguousarray(Wk[512 * g : 512 * (g + 1), :].T.astype(bf)) for g in range(2)]
    wvTs = [np.ascontiguousarray(Wv[512 * g : 512 * (g + 1), :].T.astype(bf)) for g in range(2)]
    woTs = [np.ascontiguousarray(Wo[:, 1024 * g : 1024 * (g + 1)].T.astype(bf)) for g in range(2)]

    in_maps = []
    for c in range(8):
        b, g = c // 2, c % 2
        im = {
            "xT": xTs[b],
            "wqT": wqTs[g],
            "wkT": wkTs[g],
            "wvT": wvTs[g],
            "woT": woTs[g],
        }
        im.update(consts)
        in_maps.append(im)

    res = run_bass_kernel_spmd(nc, in_maps, core_ids=list(range(8)), trace=_trace)

    y = np.empty((B, T, C), dtype=np.float32)
    for b in range(B):
        y[b] = (
            res.results[2 * b]["yT"].astype(np.float32)
            + res.results[2 * b + 1]["yT"].astype(np.float32)
        ).T
    if _trace:
        return y, res
    return y
